# revision 35
# baseline (speedup 1.0000x reference)
"""ClsAttention pooling kernel for 8 TRN2 NeuronCores.

reference:
    att_logits = einsum('bch,nc->bnh', feats, W)      # [B, N, HW]
    att_maps   = softmax(att_logits, axis=2)          # softmax over HW
    cls_feats  = einsum('bnh,bch->bnc', att_maps, feats)

Strategy (data-parallel over batch, 4 items per core):
  - One HBM pass over feats. Each item's [C, HW] slab is DMA-loaded with an
    f32->fp16 cast (SWDGE), then transposed on-chip to [HW, C] via the DMA
    xbar (2-byte dtype requirement is why fp16), with per-c-chunk contiguous
    output slabs so the xbar runs at its contiguous rate.
  - mm1 is weight-stationary: lhsT = W^T chunk [128c, 80], rhs = feats chunk
    [128c, 512h], PSUM accumulate over 4 c-chunks -> logits [80, 512h].
  - exp on ScalarE with accum_out giving the softmax denominator Z for free
    (exp without max-subtraction is safe: logits ~ N(0,1)).
  - E [80, HW] is xbar-transposed (tiny, 640KB) to [HW, 80] chunks used as
    the mm2 stationary operand; mm2 rhs is a strided view of the transposed
    feats slabs, N=512, accumulated over 32 h-chunks in fp32 PSUM.
  - normalize U/Z in fp32 and store.
"""

import numpy as np

import concourse.bass as bass
import concourse.mybir as mybir
import concourse.tile as tile
from concourse import bacc
from concourse.bass_utils import run_bass_kernel_spmd
from concourse.masks import make_identity

B, C, HW, NCLS = 32, 512, 4096, 80
NCORES = 8
BPC = B // NCORES  # batch items per core
CCH = C // 128     # c chunks (mm1 contraction)
HCH = HW // 128    # h chunks (mm2 contraction)
HB = 512           # h block for mm1 moving operand / PSUM bank
NHB = HW // HB     # 8 mm1 h-blocks per item
# The DMA xbar (transpose) and normal DMA copies are mutually exclusive on
# the SDMA engines (xbar_mode), so their times add. Offload some c-chunks'
# transposes to the TensorE transpose + DVE drain, which run in parallel
# with the HBM loads.
PE_CI = (0, 1, 2, 3)  # c-chunks transposed on PE+DVE
XBAR_CI = ()          # c-chunks transposed via DMA xbar
USE_XBAR_E = True     # transpose E via xbar (True) or PE (False)
CDT = mybir.dt.float16
F32 = mybir.dt.float32

_cached_nc = None


def _build():
    global _cached_nc
    if _cached_nc is not None:
        return _cached_nc
    nc = bacc.Bacc("TRN2", target_bir_lowering=False, debug=False)
    feats = nc.dram_tensor("feats", [BPC, C, HW], F32, kind="ExternalInput")
    wt = nc.dram_tensor("wt", [C, NCLS], F32, kind="ExternalInput")
    out = nc.dram_tensor("out", [BPC, NCLS, C], F32, kind="ExternalOutput")

    with tile.TileContext(nc) as tc:
        with (
            tc.tile_pool(name="singles", bufs=1) as singles,
            tc.tile_pool(name="fpool", bufs=2) as fpool,
            tc.tile_pool(name="tpool", bufs=2) as tpool,
            tc.tile_pool(name="epool", bufs=2) as epool,
            tc.tile_pool(name="etpool", bufs=2) as etpool,
            tc.tile_pool(name="zpool", bufs=2) as zpool,
            tc.tile_pool(name="opool", bufs=2) as opool,
            tc.tile_pool(name="plp", bufs=3, space="PSUM") as plp,
            tc.tile_pool(name="pup", bufs=2, space="PSUM") as pup,
            tc.tile_pool(name="ptp", bufs=3, space="PSUM") as ptp,
        ):
            wt_sb = singles.tile([128, CCH, NCLS], CDT)
            for ci in range(CCH):
                nc.gpsimd.dma_start(
                    out=wt_sb[:, ci, :], in_=wt[128 * ci : 128 * (ci + 1), :]
                )
            ident = singles.tile([128, 128], CDT)
            make_identity(nc, ident)
            # warm up the PE clock (HAM) while the first casts run
            for _ in range(10):
                wu = ptp.tile([128, 4, 128], CDT, name="wu", tag="pt")
                for t in range(4):
                    nc.tensor.transpose(wu[:, t, :], ident, ident)

            for b in range(BPC):
                # load + cast feats[b] to fp16, natural [c, h] layout
                fb = fpool.tile([128, CCH, HW], CDT)
                for ci in range(CCH):
                    nc.gpsimd.dma_start(
                        out=fb[:, ci, :], in_=feats[b, 128 * ci : 128 * (ci + 1), :]
                    )
                # mm1 (weight stationary) + exp + Z accumulation
                E = epool.tile([NCLS, HW], CDT)
                zp = zpool.tile([NCLS, NHB], F32)
                for cb in range(NHB):
                    pl = plp.tile([NCLS, HB], F32)
                    for k, ci in enumerate(range(CCH)):
                        nc.tensor.matmul(
                            pl,
                            lhsT=wt_sb[:, ci, :],
                            rhs=fb[:, ci, bass.ts(cb, HB)],
                            start=(k == 0),
                            stop=(k == CCH - 1),
                        )
                    nc.scalar.activation(
                        out=E[:, bass.ts(cb, HB)],
                        in_=pl,
                        func=mybir.ActivationFunctionType.Exp,
                        accum_out=zp[:, cb : cb + 1],
                    )
                # on-chip transpose; contiguous slab per c-chunk:
                # ftT[p, ci, hj, c] = feats^T[hj*128+p, ci*128+c]
                ftT = tpool.tile([128, CCH, HCH, 128], CDT)
                for ci in XBAR_CI:
                    nc.sync.dma_start_transpose(
                        out=ftT[:, ci, :, :], in_=fb[:, ci, :]
                    )
                for ci in PE_CI:
                    for hg in range(HCH // 4):
                        pt = ptp.tile([128, 4, 128], CDT, name="pt", tag="pt")
                        for t in range(4):
                            nc.tensor.transpose(
                                pt[:, t, :],
                                fb[:, ci, bass.ts(4 * hg + t, 128)],
                                ident,
                            )
                        nc.vector.tensor_copy(
                            out=ftT[:, ci, 4 * hg : 4 * hg + 4, :], in_=pt
                        )
                # E^T chunks for mm2 stationary: eT[p, hj, n] = E[n, hj*128+p]
                eT = etpool.tile([128, HCH, 96], CDT)
                if USE_XBAR_E:
                    nc.sync.dma_start_transpose(out=eT[:, :, 0:NCLS], in_=E)
                else:
                    for hj in range(HCH):
                        pe_ = ptp.tile([128, NCLS], CDT, name="pe_", tag="pt")
                        nc.tensor.transpose(
                            pe_, E[:, bass.ts(hj, 128)], ident[0:NCLS, 0:NCLS]
                        )
                        nc.vector.tensor_copy(out=eT[:, hj, 0:NCLS], in_=pe_)
                # Z and 1/Z
                z = zpool.tile([NCLS, 1], F32)
                nc.vector.reduce_sum(z, zp, axis=mybir.AxisListType.X)
                zr = zpool.tile([NCLS, 1], F32)
                nc.vector.reciprocal(zr, z)
                # mm2: U = E @ feats^T accumulated over h chunks
                pu = pup.tile([NCLS, C], F32)
                for hj in range(HCH):
                    nc.tensor.matmul(
                        pu,
                        lhsT=eT[:, hj, 0:NCLS],
                        rhs=ftT[:, :, hj, :],
                        start=(hj == 0),
                        stop=(hj == HCH - 1),
                    )
                # cls = U / Z
                ob = opool.tile([NCLS, C], F32)
                nc.vector.tensor_scalar_mul(ob, pu, zr)
                nc.sync.dma_start(out=out[b], in_=ob)

    nc.compile()
    _cached_nc = nc
    return nc


def kernel(feats: np.ndarray, W: np.ndarray, **run_kwargs) -> np.ndarray:
    nc = _build()
    feats = np.ascontiguousarray(np.asarray(feats), dtype=np.float32)
    wt = np.ascontiguousarray(np.asarray(W, dtype=np.float32).T)
    in_maps = [
        {"feats": np.ascontiguousarray(feats[i * BPC : (i + 1) * BPC]), "wt": wt}
        for i in range(NCORES)
    ]
    res = run_bass_kernel_spmd(nc, in_maps, list(range(NCORES)), **run_kwargs)
    out = np.concatenate([r["out"] for r in res.results], axis=0)
    if run_kwargs:
        kernel.last_results = res
    return np.asarray(out, dtype=np.float32)
